# revision 1
# baseline (speedup 1.0000x reference)
"""Trainium2 Bass kernel for a custom LSTM cell with LayerNorms.

Data-parallel across 8 NeuronCores: batch B=8192 is split into 8 shards of
1024 rows; weights are replicated. On-chip, activations are kept in
feature-major ("transposed") layout [feature, batch] so that:
  - gate matmuls take W.T chunks (PE-transposed, cast bf16) as the
    stationary operand and activation chunks (bf16) as the moving operand,
  - per-feature LayerNorm affine + nonlinearity fuse into single ScalarE
    activation ops (per-partition scale/bias),
  - per-batch LN statistics are ones-vector matmuls accumulating across
    feature chunks into one PSUM bank; mean/rstd rows are broadcast across
    partitions with gpsimd.partition_broadcast.
Gate activations are spilled to DRAM scratch and restreamed for the state
update to stay inside SBUF.
"""

import sys
from contextlib import ExitStack

import numpy as np

sys.path.insert(0, "/opt/trn_rl_repo")

import concourse.bass as bass
import concourse.tile as tile
from concourse import bacc, mybir
from concourse.bass_utils import run_bass_kernel_spmd
from concourse.masks import make_identity

F32 = mybir.dt.float32
BF16 = mybir.dt.bfloat16
AF = mybir.ActivationFunctionType

B, CIN, H = 8192, 512, 2048
NCORES = 8
BC = B // NCORES            # 1024 batch rows per core
NBT = BC // 128             # 8 batch row-tiles
H2 = 2 * H                  # 4096
KC = H2 // 128              # 32 contraction chunks for gate matmuls
FC = H // 128               # 16 feature chunks per gate output
PC = CIN // 128             # 4 contraction chunks for the input projection
NHB = BC // 512             # 2 PSUM batch halves (N=512 each)

GATES = ("f", "i", "c2", "o")
_PHASE_LIMIT = "full"   # profiling hook: ln | gate_f | gates | cell | full
GATE_FUNC = {"f": AF.Sigmoid, "i": AF.Sigmoid, "c2": AF.Tanh, "o": AF.Sigmoid}


def _bcast_row(row_ap, parts=128):
    """Partition-broadcast view of a [1, N] DRAM AP."""
    return bass.AP(
        tensor=row_ap.tensor,
        offset=row_ap.offset,
        ap=[[0, parts]] + [list(d) for d in row_ap.ap[1:]],
    )


def build_kernel(nc):
    ins = {}

    def din(name, shape):
        ins[name] = nc.dram_tensor(name, shape, F32, kind="ExternalInput").ap()

    din("x", (BC, 1, CIN))
    din("h", (BC, H))
    din("c", (BC, H))
    din("W_proj", (H, CIN))
    din("b_proj", (H,))
    din("g_ln", (H2,))
    din("b_ln", (H2,))
    din("g_cn", (H,))
    din("b_cn", (H,))
    din("g_hn", (H,))
    din("b_hn", (H,))
    for g in GATES:
        din(f"W_{g}", (H, H2))
        din(f"b_{g}", (H,))
        din(f"g_{g}", (H,))
        din(f"beta_{g}", (H,))

    out_h = nc.dram_tensor("out_h", (BC, H), F32, kind="ExternalOutput").ap()
    out_c = nc.dram_tensor("out_c", (BC, H), F32, kind="ExternalOutput").ap()

    with tile.TileContext(nc) as tc, ExitStack() as ctx:
        build_body(ctx, tc, ins, out_h, out_c)
    nc.compile()
    return nc


def build_body(ctx, tc, ins, out_h, out_c):
    nc = tc.nc
    global _PHASE_LIMIT

    singles = ctx.enter_context(tc.tile_pool(name="singles", bufs=1))
    stage = ctx.enter_context(tc.tile_pool(name="stage", bufs=4))
    wt_pool = ctx.enter_context(tc.tile_pool(name="wt", bufs=12))
    rows = ctx.enter_context(tc.tile_pool(name="rows", bufs=1))
    bcasts = ctx.enter_context(tc.tile_pool(name="bcasts", bufs=2))
    scratch = ctx.enter_context(tc.tile_pool(name="scratch", bufs=2))
    sq_pool = ctx.enter_context(tc.tile_pool(name="sq", bufs=2))
    tpsum = ctx.enter_context(tc.tile_pool(name="tpsum", bufs=1, space="PSUM"))
    mm_psum = ctx.enter_context(tc.tile_pool(name="mmpsum", bufs=5, space="PSUM"))
    st_psum = ctx.enter_context(tc.tile_pool(name="stpsum", bufs=2, space="PSUM"))
    dram = ctx.enter_context(tc.tile_pool(name="dram", bufs=1, space="DRAM"))

    ident = singles.tile([128, 128], F32)
    make_identity(nc, ident)
    ones_bf = singles.tile([128, 1], BF16)
    nc.vector.memset(ones_bf, 1.0)
    ones_f32 = singles.tile([128, 1], F32)
    nc.vector.memset(ones_f32, 1.0)
    eps_row = singles.tile([1, 1], F32)
    nc.vector.memset(eps_row, 1e-5)

    # Per-feature constants in chunk-column layout [128, n_chunks]:
    # element (p, c) = v[c*128 + p].
    def load_cols(name, n_chunks):
        t = singles.tile([128, n_chunks], F32, name=f"cols_{name}")
        nc.sync.dma_start(out=t, in_=ins[name].rearrange("(c p) -> p c", p=128))
        return t

    g_ln = load_cols("g_ln", KC)
    b_ln = load_cols("b_ln", KC)
    g_cn = load_cols("g_cn", FC)
    b_cn = load_cols("b_cn", FC)
    g_hn = load_cols("g_hn", FC)
    b_hn = load_cols("b_hn", FC)
    b_proj = load_cols("b_proj", FC)
    gate_g = {g: load_cols(f"g_{g}", FC) for g in GATES}
    gate_beta = {g: load_cols(f"beta_{g}", FC) for g in GATES}
    gate_b = {g: load_cols(f"b_{g}", FC) for g in GATES}

    def transpose_chunk(src_ap, dst_ap):
        """PE-transpose a [128,128] fp32 SBUF block into dst (casts via copy)."""
        pt = tpsum.tile([128, 128], F32, tag="tp")
        nc.tensor.transpose(pt, src_ap, ident)
        nc.vector.tensor_copy(out=dst_ap, in_=pt)

    # The four stats accumulation chains (sum z / sum z^2 per batch half)
    # share one PSUM bank at quadrant partitions 0/32/64/96 (walrus only
    # accepts matmul outputs at 32-aligned base partitions).
    ROFF = (0, 32, 64, 96)

    def stats_mm(stats, chunk, first, last, ones):
        for hb in range(NHB):
            zs = chunk[:, bass.ts(hb, 512)]
            sq = sq_pool.tile([128, 512], chunk.dtype, tag="sq")
            nc.scalar.square(sq, zs)
            r0, r1 = ROFF[2 * hb], ROFF[2 * hb + 1]
            nc.tensor.matmul(stats[r0 : r0 + 1, :], ones, zs,
                             start=first, stop=last, tile_position=(0, r0))
            nc.tensor.matmul(stats[r1 : r1 + 1, :], ones, sq,
                             start=first, stop=last, tile_position=(0, r1))

    def stats_to_bcast(stats, d):
        """[4,512] stats PSUM -> broadcast tiles (a_bc, c_bc) [128, BC] such
        that z_norm = z * a_bc + c_bc."""
        m = rows.tile([1, BC], F32, tag="mrow")
        v = rows.tile([1, BC], F32, tag="vrow")
        for hb in range(NHB):
            s = bass.ts(hb, 512)
            r0, r1 = ROFF[2 * hb], ROFF[2 * hb + 1]
            nc.vector.tensor_scalar_mul(m[:, s], stats[r0 : r0 + 1, :], 1.0 / d)
            nc.vector.tensor_scalar_mul(v[:, s], stats[r1 : r1 + 1, :], 1.0 / d)
        msq = rows.tile([1, BC], F32, tag="msq")
        nc.vector.tensor_mul(msq, m, m)
        nc.vector.tensor_sub(v, v, msq)          # v = var
        nc.scalar.activation(out=v, in_=v, func=AF.Sqrt, bias=eps_row, scale=1.0)
        nc.vector.reciprocal(out=v, in_=v)       # v = rstd
        nc.vector.tensor_mul(msq, m, v)
        nc.vector.tensor_scalar_mul(msq, msq, -1.0)  # msq = -m*rstd
        # Broadcast across partitions via a DRAM roundtrip (stride-0
        # partition APs are only legal with a DRAM source).
        a_bc = bcasts.tile([128, BC], F32, tag="abc")
        c_bc = bcasts.tile([128, BC], F32, tag="cbc")
        for row, bc in ((v, a_bc), (msq, c_bc)):
            drow = dram.tile([1, BC], F32, name="drow", tag="drow", bufs=4)
            nc.sync.dma_start(out=drow, in_=row)
            nc.sync.dma_start(out=bc, in_=_bcast_row(drow))
        return a_bc, c_bc

    def apply_ln(z_chunk, a_bc, c_bc, g_cols, b_cols, fc, func, dst):
        """dst = func((z*a_bc + c_bc) * g[:,fc] + b[:,fc])"""
        t = scratch.tile([128, BC], F32, tag="apply")
        nc.vector.tensor_mul(t, z_chunk, a_bc)
        nc.vector.tensor_add(t, t, c_bc)
        nc.scalar.activation(out=dst, in_=t, func=func,
                             scale=g_cols[:, fc : fc + 1],
                             bias=b_cols[:, fc : fc + 1])

    # ---- Phase 0: cast all weights to bf16 in DRAM scratch ----------------
    # Enables XBAR DMA-transposed weight loads (2-byte dtypes only), which
    # replaces per-chunk PE transposes + DVE copybacks entirely.
    wbf = {"proj": dram.tile([H, CIN], BF16, name="wbf_proj")}
    for g in GATES:
        wbf[g] = dram.tile([H, H2], BF16, name=f"wbf_{g}")

    def cast_weight(dst, src, cols):
        for fc in range(FC):
            for q in range(cols // 1024):
                ws = stage.tile([128, 1024], F32, tag="cast1024", name="cws", bufs=2)
                nc.gpsimd.dma_start(
                    out=ws, in_=src[bass.ts(fc, 128), bass.ts(q, 1024)])
                wb = stage.tile([128, 1024], BF16, tag="wbf", name="cwb", bufs=3)
                nc.gpsimd.tensor_copy(out=wb, in_=ws)
                nc.gpsimd.dma_start(
                    out=dst[bass.ts(fc, 128), bass.ts(q, 1024)], in_=wb)

    def cast_weight_512(dst, src):
        for fc in range(FC):
            ws = stage.tile([128, 512], F32, tag="cast512", name="cws5", bufs=2)
            nc.gpsimd.dma_start(out=ws, in_=src[bass.ts(fc, 128), :])
            wb = stage.tile([128, 512], BF16, tag="wbf5", name="cwb5", bufs=2)
            nc.gpsimd.tensor_copy(out=wb, in_=ws)
            nc.gpsimd.dma_start(out=dst[bass.ts(fc, 128), :], in_=wb)

    cast_weight_512(wbf["proj"], ins["W_proj"])
    for g in GATES:
        cast_weight(wbf[g], ins[f"W_{g}"], H2)

    # ---- Phase 1: load + transpose x and h --------------------------------
    # zg is allocated below comb on the pool stack: comb releases after the
    # gate matmuls while zg (holding gate o's activations in place) lives
    # through the state phase.
    zg_pool = tc.alloc_tile_pool(name="zg", bufs=1)
    comb_pool = tc.alloc_tile_pool(name="comb", bufs=1)
    xT_pool = tc.alloc_tile_pool(name="xTp", bufs=1)

    comb = [comb_pool.tile([128, BC], BF16, name=f"comb{c}", tag=f"comb{c}")
            for c in range(KC)]
    xT = [xT_pool.tile([128, BC], BF16, name=f"xT{k}", tag=f"xT{k}")
          for k in range(PC)]

    x2d = ins["x"].rearrange("b one k -> (b one) k")
    for bt in range(NBT):
        xs = stage.tile([128, 512], F32, tag="stg512")
        nc.scalar.dma_start(out=xs, in_=x2d[bass.ts(bt, 128), :])
        for j in range(PC):
            transpose_chunk(xs[:, bass.ts(j, 128)], xT[j][:, bass.ts(bt, 128)])
        for half in range(2):
            hs = stage.tile([128, 1024], F32, tag="stg1024", bufs=2)
            nc.scalar.dma_start(
                out=hs, in_=ins["h"][bass.ts(bt, 128), bass.ts(half, 1024)])
            for j in range(FC // 2):
                fc = half * (FC // 2) + j
                transpose_chunk(hs[:, bass.ts(j, 128)],
                                comb[FC + fc][:, bass.ts(bt, 128)])

    # ---- Phase 2: input projection xp^T = W_proj @ x^T + b_proj -----------
    # Feature chunks are processed in pairs: one XBAR-transposed weight load
    # [128k, 256f] feeds two PSUM accumulation chains (4 banks with NHB=2).
    comb_stats = st_psum.tile([128, 512], F32, tag="stats")

    def mm_block(dst_chunks, wsrc, xsrc, nk, bias_cols):
        """dst_chunks[f][128, BC] (bf16) = wsrc.T-chunks @ xsrc + bias.
        Feature chunks iterate in pairs. LN stats matmuls are deferred by
        the caller so the PE stream here is pure back-to-back matmuls."""
        nfc = len(dst_chunks)
        for fg in range(nfc // 2):
            zp = [[mm_psum.tile([128, 512], F32, tag="zpsum", name="zp")
                   for _ in range(NHB)] for _ in range(2)]
            for k in range(nk):
                wt = wt_pool.tile([128, 256], BF16, tag="wt")
                nc.sync.dma_start_transpose(
                    wt, wsrc[bass.ts(fg, 256), bass.ts(k, 128)])
                for f in range(2):
                    for hb in range(NHB):
                        nc.tensor.matmul(
                            zp[f][hb], wt[:, bass.ts(f, 128)],
                            xsrc[k][:, bass.ts(hb, 512)],
                            start=(k == 0), stop=(k == nk - 1))
            for f in range(2):
                fc = 2 * fg + f
                for hb in range(NHB):
                    nc.vector.tensor_scalar_add(
                        out=dst_chunks[fc][:, bass.ts(hb, 512)],
                        in0=zp[f][hb], scalar1=bias_cols[:, fc : fc + 1])

    if _PHASE_LIMIT == "prep":
        xT_pool.release(); comb_pool.release(); zg_pool.release()
        return
    mm_block(comb[:FC], wbf["proj"], xT, PC, b_proj)
    for fc in range(FC):
        stats_mm(comb_stats, comb[fc], first=(fc == 0), last=False,
                 ones=ones_bf)
    for j in range(FC):
        stats_mm(comb_stats, comb[FC + j], first=False, last=(j == FC - 1),
                 ones=ones_bf)
    xT_pool.release()

    # ---- Phase 3: combined LayerNorm + tanh (in place) --------------------
    a_bc, c_bc = stats_to_bcast(comb_stats, float(H2))
    for c in range(KC):
        apply_ln(comb[c], a_bc, c_bc, g_ln, b_ln, c, AF.Tanh, comb[c])

    if _PHASE_LIMIT == "ln":
        comb_pool.release(); zg_pool.release()
        return
    # ---- Phase 4: gates z = W_g @ comb + b_g; LN; sigmoid/tanh ------------
    # f, i, c2 activations spill to DRAM and restream in phase 5; gate o's
    # activations stay resident in the zg tiles for phase 6.
    act_dram = {g: dram.tile([H, BC], BF16, name=f"act_{g}")
                for g in GATES if g != "o"}
    o_act = None
    for g in GATES:
        wg = ins[f"W_{g}"]
        stats = st_psum.tile([128, 512], F32, tag="stats")
        zg = [zg_pool.tile([128, BC], BF16, name=f"z_{g}{fc}", tag=f"zg{fc}")
              for fc in range(FC)]
        mm_block(zg, wbf[g], comb, KC, gate_b[g])
        for fc in range(FC):
            stats_mm(stats, zg[fc], first=(fc == 0), last=(fc == FC - 1),
                     ones=ones_bf)
        a_bc, c_bc = stats_to_bcast(stats, float(H))
        for fc in range(FC):
            apply_ln(zg[fc], a_bc, c_bc, gate_g[g], gate_beta[g], fc,
                     GATE_FUNC[g], zg[fc])
            if g != "o":
                nc.scalar.dma_start(out=act_dram[g][bass.ts(fc, 128), :],
                                    in_=zg[fc])
        if g == "o":
            o_act = zg
        if _PHASE_LIMIT == "gate_f":
            comb_pool.release(); zg_pool.release()
            return

    if _PHASE_LIMIT == "gates":
        comb_pool.release(); zg_pool.release()
        return
    comb_pool.release()

    # ---- Phase 5: cell update cp = f*c + i*cc; next_cell = LN_cn(cp) ------
    state = tc.alloc_tile_pool(name="state", bufs=1)
    gs_pool = tc.alloc_tile_pool(name="gstream", bufs=2)
    asm_pool = tc.alloc_tile_pool(name="asm", bufs=2)

    cp = [state.tile([128, BC], F32, name=f"cp{j}", tag=f"cpf{j}")
          for j in range(FC)]
    cn_stats = st_psum.tile([128, 512], F32, tag="stats")
    for fc in range(FC):
        cT = gs_pool.tile([128, BC], BF16, tag="cT", bufs=2)
        for bt in range(NBT):
            cs = stage.tile([128, 128], F32, tag="stg128")
            nc.scalar.dma_start(out=cs,
                              in_=ins["c"][bass.ts(bt, 128), bass.ts(fc, 128)])
            transpose_chunk(cs, cT[:, bass.ts(bt, 128)])
        fa = gs_pool.tile([128, BC], BF16, tag="fstream", bufs=2)
        ia = gs_pool.tile([128, BC], BF16, tag="istream", bufs=2)
        ca = gs_pool.tile([128, BC], BF16, tag="cstream", bufs=2)
        nc.scalar.dma_start(out=fa, in_=act_dram["f"][bass.ts(fc, 128), :])
        nc.scalar.dma_start(out=ia, in_=act_dram["i"][bass.ts(fc, 128), :])
        nc.scalar.dma_start(out=ca, in_=act_dram["c2"][bass.ts(fc, 128), :])
        t = scratch.tile([128, BC], F32, tag="apply")
        nc.vector.tensor_mul(t, fa, cT)
        nc.vector.tensor_mul(cp[fc], ia, ca)
        nc.vector.tensor_add(cp[fc], cp[fc], t)
        stats_mm(cn_stats, cp[fc], first=(fc == 0), last=(fc == FC - 1),
                 ones=ones_f32)

    a_bc, c_bc = stats_to_bcast(cn_stats, float(H))
    hn_stats = st_psum.tile([128, 512], F32, tag="stats")
    hp = []
    for fc in range(FC):
        apply_ln(cp[fc], a_bc, c_bc, g_cn, b_cn, fc, AF.Identity, cp[fc])

    # write next_cell (transpose back to batch-major), then hidden path
    for bt in range(NBT):
        for hh in range(2):
            asm = asm_pool.tile([128, H // 2], F32, tag="asm", bufs=1)
            for j in range(FC // 2):
                fc = hh * (FC // 2) + j
                transpose_chunk(cp[fc][:, bass.ts(bt, 128)],
                                asm[:, bass.ts(j, 128)])
            nc.scalar.dma_start(
                out=out_c[bass.ts(bt, 128), bass.ts(hh, H // 2)], in_=asm)

    if _PHASE_LIMIT == "cell":
        asm_pool.release(); gs_pool.release(); state.release(); zg_pool.release()
        return
    # ---- Phase 6: hidden hp = o * tanh(next_cell); LN + tanh --------------
    for fc in range(FC):
        tcell = state.tile([128, BC], BF16, tag="tcell", bufs=2)
        nc.scalar.activation(out=tcell, in_=cp[fc], func=AF.Tanh)
        hpt = state.tile([128, BC], F32, name=f"hp{fc}", tag=f"cpf{fc}")
        nc.vector.tensor_mul(hpt, o_act[fc], tcell)
        hp.append(hpt)
        stats_mm(hn_stats, hpt, first=(fc == 0), last=(fc == FC - 1),
                 ones=ones_f32)

    a_bc, c_bc = stats_to_bcast(hn_stats, float(H))
    for fc in range(FC):
        apply_ln(hp[fc], a_bc, c_bc, g_hn, b_hn, fc, AF.Tanh, hp[fc])

    for bt in range(NBT):
        for hh in range(2):
            asm = asm_pool.tile([128, H // 2], F32, tag="asm", bufs=1)
            for j in range(FC // 2):
                fc = hh * (FC // 2) + j
                transpose_chunk(hp[fc][:, bass.ts(bt, 128)],
                                asm[:, bass.ts(j, 128)])
            nc.scalar.dma_start(
                out=out_h[bass.ts(bt, 128), bass.ts(hh, H // 2)], in_=asm)

    asm_pool.release()
    gs_pool.release()
    state.release()
    zg_pool.release()


_NC_CACHE = {}


def _get_nc():
    if "nc" not in _NC_CACHE:
        nc = bacc.Bacc(
            "TRN2",
            target_bir_lowering=False,
            debug=False,
            enable_asserts=False,
            num_devices=NCORES,
        )
        _NC_CACHE["nc"] = build_kernel(nc)
    return _NC_CACHE["nc"]


def run(inputs, **kw):
    nc = _get_nc()
    full = {k: np.ascontiguousarray(np.asarray(v, dtype=np.float32))
            for k, v in inputs.items()}
    in_maps = []
    for i in range(NCORES):
        s = slice(i * BC, (i + 1) * BC)
        m = {k: (np.ascontiguousarray(v[s]) if k in ("x", "h", "c") else v)
             for k, v in full.items()}
        in_maps.append(m)
    res = run_bass_kernel_spmd(nc, in_maps, core_ids=list(range(NCORES)), **kw)
    nh = np.concatenate([r["out_h"] for r in res.results], axis=0)
    ncl = np.concatenate([r["out_c"] for r in res.results], axis=0)
    return np.stack([nh, ncl]).astype(np.float32), res


def kernel(**inputs) -> np.ndarray:
    out, _ = run(inputs)
    return out



# revision 12
# speedup vs baseline: 2.6248x; 2.6248x over previous
"""Trainium2 Bass kernel for a custom LSTM cell with LayerNorms.

Data-parallel across 8 NeuronCores: batch B=8192 is split into 8 shards of
1024 rows; weights are replicated.

Dataflow (v2):
  - comb = tanh(LN([x W_proj^T ; h])) is built feature-major ([feature,
    batch] tiles) exactly once: x/h/W_proj are transposed on the PE, the
    concat-LN statistics are ones-vector matmuls accumulated in one PSUM
    bank, and the mean/rstd rows are broadcast via a DRAM roundtrip.
  - The four gate matmuls produce BATCH-major outputs: the stationary
    operand is a [128k, 128b] slice of comb, the moving operand is a
    [128k, 512f] slice of W^T obtained by XBAR DMA-transpose from a bf16
    copy of W (written once by a fp32->bf16 cast-during-DMA on the SWDGE
    path, chunked and emitted one gate ahead so casts overlap matmuls).
    k is the outer loop so all 8 batch-chunk PSUM banks accumulate in
    parallel and only a handful of W^T tiles are resident.
  - Batch-major layout makes every per-batch LayerNorm a free-dim problem:
    bn_stats/bn_aggr on the DVE produce mean/var per partition, the affine
    is a per-partition scalar-engine activation, and the per-feature
    gamma/beta are elementwise with partition-broadcast rows.  No stats
    matmuls, no broadcast roundtrips, no activation spills, and the
    cell/hidden state updates plus output stores need no transposes.
"""

import sys
from contextlib import ExitStack

import numpy as np

sys.path.insert(0, "/opt/trn_rl_repo")

import concourse.bass as bass
import concourse.tile as tile
from concourse import bacc, mybir
from concourse.bass_utils import run_bass_kernel_spmd
from concourse.masks import make_identity

F32 = mybir.dt.float32
BF16 = mybir.dt.bfloat16
AF = mybir.ActivationFunctionType
ALU = mybir.AluOpType

B, CIN, H = 8192, 512, 2048
NCORES = 8
BC = B // NCORES            # 1024 batch rows per core
NB = BC // 128              # 8 batch chunks
H2 = 2 * H                  # 4096
KC = H2 // 128              # 32 contraction chunks for gate matmuls
PC = CIN // 128             # 4 contraction chunks for the input projection
FC = H // 128               # 16 feature chunks (feature-major comb halves)
SW = 4                      # f sweeps per gate
FS = H // SW                # 512 features per sweep (= 1 PSUM bank)
NHB = BC // 512             # 2 PSUM batch halves for the projection

GATES = ("c2", "i", "f", "o")
GATE_FUNC = {"f": AF.Sigmoid, "i": AF.Sigmoid, "c2": AF.Tanh, "o": AF.Sigmoid}
# z-tile tag ring: c2/f share one set of buffers, i/o the other.
ZTAG = {"c2": "zE", "i": "zO", "f": "zE", "o": "zO"}
NEXT_GATE = {"c2": "i", "i": "f", "f": "o", "o": None}


def _row(ap):
    """View a 1-D [N] DRAM AP as [1, N]."""
    return ap.rearrange("(o k) -> o k", o=1)


def _bcast_row(row_ap, parts=128):
    """Partition-broadcast view of a [1, N] DRAM AP."""
    return bass.AP(
        tensor=row_ap.tensor,
        offset=row_ap.offset,
        ap=[[0, parts]] + [list(d) for d in row_ap.ap[1:]],
    )


def build_kernel(nc):
    ins = {}

    def din(name, shape):
        ins[name] = nc.dram_tensor(name, shape, F32, kind="ExternalInput").ap()

    din("x", (BC, 1, CIN))
    din("h", (BC, H))
    din("c", (BC, H))
    din("W_proj", (H, CIN))
    din("b_proj", (H,))
    din("g_ln", (H2,))
    din("b_ln", (H2,))
    din("g_cn", (H,))
    din("b_cn", (H,))
    din("g_hn", (H,))
    din("b_hn", (H,))
    for g in GATES:
        din(f"W_{g}", (H, H2))
        din(f"b_{g}", (H,))
        din(f"g_{g}", (H,))
        din(f"beta_{g}", (H,))

    out_h = nc.dram_tensor("out_h", (BC, H), F32, kind="ExternalOutput").ap()
    out_c = nc.dram_tensor("out_c", (BC, H), F32, kind="ExternalOutput").ap()

    with tile.TileContext(nc) as tc, ExitStack() as ctx:
        build_body(ctx, tc, ins, out_h, out_c)
    nc.compile()
    return nc


def build_body(ctx, tc, ins, out_h, out_c):
    nc = tc.nc

    # ---------------- deep pools (live through gates and tail) ------------
    singles = ctx.enter_context(tc.tile_pool(name="singles", bufs=1))
    smallp = ctx.enter_context(tc.tile_pool(name="smallp", bufs=1))
    tscr = ctx.enter_context(tc.tile_pool(name="tscr", bufs=1))
    cpool = ctx.enter_context(tc.tile_pool(name="cpool", bufs=1))
    bnp = ctx.enter_context(tc.tile_pool(name="bnp", bufs=1))
    dram = ctx.enter_context(tc.tile_pool(name="dram", bufs=1, space="DRAM"))

    combp = tc.alloc_tile_pool(name="comb", bufs=1)
    comb = [combp.tile([128, BC], BF16, name=f"comb{k}", tag=f"comb{k}")
            for k in range(KC)]

    ident = singles.tile([128, 128], F32)
    make_identity(nc, ident)
    ones_bf = singles.tile([128, 1], BF16)
    nc.vector.memset(ones_bf, 1.0)
    eps_col = singles.tile([128, 1], F32)
    nc.vector.memset(eps_col, 1e-5)
    eps_row = singles.tile([1, 1], F32)
    nc.vector.memset(eps_row, 1e-5)

    def load_cols(name, n):
        t = singles.tile([128, n], F32, name=f"cols_{name}")
        nc.sync.dma_start(out=t, in_=ins[name].rearrange("(c p) -> p c", p=128))
        return t

    g_ln = load_cols("g_ln", KC)
    b_ln = load_cols("b_ln", KC)
    b_proj = load_cols("b_proj", FC)

    # ---- weight casts fp32 -> bf16, DRAM -> DRAM on the SWDGE path -------
    # Only gate c2's weights are cast upfront; each later gate's casts are
    # emitted during the previous gate so the gpsimd DMA queue stays short
    # for the per-gate bias/gamma/beta row loads.
    wbf = {g: dram.tile([H, H2], BF16, name=f"wbf_{g}") for g in GATES}

    def emit_wcast(g):
        for s in range(SW):
            nc.gpsimd.dma_start(out=wbf[g][bass.ts(s, FS), :],
                                in_=ins[f"W_{g}"][bass.ts(s, FS), :])

    emit_wcast("c2")

    # ---------------- prep: x^T, h^T, W_proj^T, proj, concat-LN -----------
    prep = tc.alloc_tile_pool(name="prep", bufs=1)
    ppsum = tc.alloc_tile_pool(name="ppsum", bufs=1, space="PSUM")

    def transpose_chunk(src_ap, dst_ap):
        pt = ppsum.tile([128, 128], F32, tag="tp", bufs=2)
        nc.tensor.transpose(pt, src_ap, ident)
        nc.vector.tensor_copy(out=dst_ap, in_=pt)

    xT = [prep.tile([128, BC], BF16, name=f"xT{j}", tag=f"xT{j}")
          for j in range(PC)]
    x2d = ins["x"].rearrange("b one k -> (b one) k")
    for bt in range(NB):
        xs = prep.tile([128, CIN], F32, tag="xstage", bufs=2)
        nc.scalar.dma_start(out=xs, in_=x2d[bass.ts(bt, 128), :])
        for j in range(PC):
            transpose_chunk(xs[:, bass.ts(j, 128)], xT[j][:, bass.ts(bt, 128)])
        for half in range(2):
            hs = prep.tile([128, 1024], F32, tag="hstage", bufs=2)
            nc.sync.dma_start(
                out=hs, in_=ins["h"][bass.ts(bt, 128), bass.ts(half, 1024)])
            for j in range(8):
                transpose_chunk(hs[:, bass.ts(j, 128)],
                                comb[FC + half * 8 + j][:, bass.ts(bt, 128)])

    wpT = [prep.tile([128, H], BF16, name=f"wpT{j}", tag=f"wpT{j}")
           for j in range(PC)]
    for f in range(FC):
        ws = prep.tile([128, CIN], F32, tag="wpstage", bufs=2)
        nc.scalar.dma_start(out=ws, in_=ins["W_proj"][bass.ts(f, 128), :])
        for j in range(PC):
            transpose_chunk(ws[:, bass.ts(j, 128)], wpT[j][:, bass.ts(f, 128)])

    # xp^T = W_proj @ x^T + b_proj, feature-major into comb[0..FC)
    for f in range(FC):
        pj = [ppsum.tile([128, 512], F32, name=f"pj{f}_{hb}",
                         tag=f"pj{f % 2}_{hb}", bufs=1)
              for hb in range(NHB)]
        for j in range(PC):
            for hb in range(NHB):
                nc.tensor.matmul(pj[hb], wpT[j][:, bass.ts(f, 128)],
                                 xT[j][:, bass.ts(hb, 512)],
                                 start=(j == 0), stop=(j == PC - 1))
        for hb in range(NHB):
            nc.vector.tensor_scalar_add(out=comb[f][:, bass.ts(hb, 512)],
                                        in0=pj[hb], scalar1=b_proj[:, f:f + 1])

    # concat-LN stats: per-batch sum(z), sum(z^2) via ones-matmuls into one
    # PSUM bank (quadrant rows 0/32/64/96).
    ROFF = (0, 32, 64, 96)
    cstat = ppsum.tile([128, 512], F32, tag="stats")
    for k in range(KC):
        for hb in range(NHB):
            zs = comb[k][:, bass.ts(hb, 512)]
            sq = prep.tile([128, 512], BF16, tag="sq", bufs=2)
            nc.scalar.square(sq, zs)
            r0, r1 = ROFF[2 * hb], ROFF[2 * hb + 1]
            nc.tensor.matmul(cstat[r0:r0 + 1, :], ones_bf, zs,
                             start=(k == 0), stop=(k == KC - 1),
                             tile_position=(0, r0))
            nc.tensor.matmul(cstat[r1:r1 + 1, :], ones_bf, sq,
                             start=(k == 0), stop=(k == KC - 1),
                             tile_position=(0, r1))

    m = prep.tile([1, BC], F32, tag="mrow")
    v = prep.tile([1, BC], F32, tag="vrow")
    msq = prep.tile([1, BC], F32, tag="msqrow")
    for hb in range(NHB):
        s = bass.ts(hb, 512)
        r0, r1 = ROFF[2 * hb], ROFF[2 * hb + 1]
        nc.vector.tensor_scalar_mul(m[:, s], cstat[r0:r0 + 1, :], 1.0 / H2)
        nc.vector.tensor_scalar_mul(v[:, s], cstat[r1:r1 + 1, :], 1.0 / H2)
    nc.vector.tensor_mul(msq, m, m)
    nc.vector.tensor_sub(v, v, msq)                       # var
    nc.scalar.activation(out=v, in_=v, func=AF.Sqrt, bias=eps_row, scale=1.0)
    nc.vector.reciprocal(out=v, in_=v)                    # rstd
    nc.vector.tensor_mul(msq, m, v)
    nc.vector.tensor_scalar_mul(msq, msq, -1.0)           # -mean*rstd
    # Broadcast across partitions via a DRAM roundtrip on the HWDGE rings
    # (keeps the gpsimd queue free for weight-cast traffic).
    a_bc = prep.tile([128, BC], F32, tag="abc")
    c_bc = prep.tile([128, BC], F32, tag="cbc")
    for row, bc in ((v, a_bc), (msq, c_bc)):
        drow = dram.tile([1, BC], F32, name="drow", tag="drow", bufs=4)
        nc.sync.dma_start(out=drow, in_=row)
        nc.sync.dma_start(out=bc, in_=_bcast_row(drow))
    for k in range(KC):
        t = prep.tile([128, BC], F32, tag="apply", bufs=4)
        nc.vector.tensor_mul(t, comb[k], a_bc)
        nc.vector.tensor_add(t, t, c_bc)
        nc.scalar.activation(out=comb[k], in_=t, func=AF.Tanh,
                             scale=g_ln[:, k:k + 1], bias=b_ln[:, k:k + 1])

    ppsum.release()
    prep.release()

    # ---------------- gates: batch-major z = comb^T @ W^T ------------------
    zpool = tc.alloc_tile_pool(name="zpool", bufs=1)
    wtp = tc.alloc_tile_pool(name="wtp", bufs=1)
    vbc = tc.alloc_tile_pool(name="vbc", bufs=1)
    gpsum = tc.alloc_tile_pool(name="gpsum", bufs=1, space="PSUM")

    def bcast_vec(pool, name, tag):
        """[H] DRAM fp32 row -> [128, H] bf16 partition-broadcast tile."""
        row = pool.tile([1, H], BF16, name=f"row_{name}", tag="vrow", bufs=2)
        nc.gpsimd.dma_start(out=row, in_=_row(ins[name]))  # cast f32->bf16
        full = pool.tile([128, H], BF16, name=f"bc_{name}", tag=tag, bufs=1)
        nc.gpsimd.partition_broadcast(full, row)
        return full

    def bm_norm_cols(bn_t):
        """bn groups -> (rstd, -mean*rstd) per-partition columns."""
        mv = smallp.tile([128, 2], F32, tag="mv", bufs=8)
        nc.vector.bn_aggr(mv, bn_t)
        sd = smallp.tile([128, 1], F32, tag="sd", bufs=8)
        nc.scalar.activation(out=sd, in_=mv[:, 1:2], func=AF.Sqrt,
                             bias=eps_col, scale=1.0)
        rstd = smallp.tile([128, 1], F32, tag="rstd", bufs=8)
        nc.vector.reciprocal(rstd, sd)
        negm = smallp.tile([128, 1], F32, tag="negm", bufs=8)
        nc.vector.tensor_scalar(out=negm, in0=mv[:, 0:1], scalar1=rstd,
                                scalar2=-1.0, op0=ALU.mult, op1=ALU.mult)
        return rstd, negm

    def bm_apply_slice(dst_ap, src_ap, rstd, negm, g_bc_s, b_bc_s, func):
        """dst = func(((src - m)*rstd)*g + b) on one [128, FS] slice."""
        t = tscr.tile([128, FS], BF16, tag="t", bufs=4)
        nc.scalar.activation(out=t, in_=src_ap, func=AF.Identity,
                             scale=rstd, bias=negm)
        nc.vector.tensor_mul(t, t, g_bc_s)
        nc.vector.tensor_add(t, t, b_bc_s)
        nc.scalar.activation(out=dst_ap, in_=t, func=func)

    zt = {}
    cp = [None] * NB
    for g in GATES:
        bb = bcast_vec(vbc, f"b_{g}", "bb")
        gg = bcast_vec(vbc, f"g_{g}", "gg")
        tb = bcast_vec(vbc, f"beta_{g}", "tb")

        # prefetch: c tiles before gate o's casts; next gate's weight casts
        if g == "f":
            pass  # c loads are emitted in the f-hook below, before o's casts
        if NEXT_GATE[g] is not None and g != "f":
            emit_wcast(NEXT_GATE[g])

        z = [zpool.tile([128, H], BF16, name=f"z_{g}{b}", tag=f"{ZTAG[g]}{b}")
             for b in range(NB)]
        bn = [bnp.tile([128, 6 * SW], F32, name=f"bn_{g}{b}", tag=f"bn{b}",
                       bufs=2)
              for b in range(NB)]

        for s in range(SW):
            ps = [gpsum.tile([128, FS], F32, name=f"ps_{g}{s}_{b}",
                             tag=f"mm{b}", bufs=1)
                  for b in range(NB)]
            for k in range(KC):
                wt = wtp.tile([128, FS], BF16, tag="wt", bufs=6)
                nc.sync.dma_start_transpose(
                    wt, wbf[g][bass.ts(s, FS), bass.ts(k, 128)])
                for b in range(NB):
                    nc.tensor.matmul(ps[b], comb[k][:, bass.ts(b, 128)], wt,
                                     start=(k == 0), stop=(k == KC - 1))
            for b in range(NB):
                # drain + bias (free-dim) in one DVE op, then stats
                zs = z[b][:, bass.ts(s, FS)]
                nc.vector.tensor_add(zs, ps[b], bb[:, bass.ts(s, FS)])
                nc.vector.bn_stats(out=bn[b][:, 6 * s:6 * (s + 1)], in_=zs)

        for b in range(NB):
            rstd, negm = bm_norm_cols(bn[b])
            for s in range(SW):
                sl = bass.ts(s, FS)
                bm_apply_slice(z[b][:, sl], z[b][:, sl], rstd, negm,
                               gg[:, sl], tb[:, sl], GATE_FUNC[g])

        zt[g] = z

        if g == "i":
            # cp = i * cc  (cc = gate c2 output, still resident)
            for b in range(NB):
                cp[b] = zpool.tile([128, H], BF16, name=f"cp{b}",
                                   tag=f"cp{b}")
                nc.vector.tensor_mul(cp[b], zt["i"][b], zt["c2"][b])
        elif g == "f":
            # cp += f * c, with c loaded batch-major (cast to bf16 in DMA);
            # then prefetch gate o's weight casts.
            for b in range(NB):
                ct = cpool.tile([128, H], BF16, tag="c", bufs=1)
                nc.gpsimd.dma_start(out=ct, in_=ins["c"][bass.ts(b, 128), :])
                for s in range(SW):
                    sl = bass.ts(s, FS)
                    t = tscr.tile([128, FS], BF16, tag="t", bufs=4)
                    nc.vector.tensor_mul(t, zt["f"][b][:, sl], ct[:, sl])
                    nc.vector.tensor_add(cp[b][:, sl], cp[b][:, sl], t)
            emit_wcast("o")

    gpsum.release()
    vbc.release()
    wtp.release()

    # ---------------- tail: cell LN, hidden path, outputs ------------------
    tailp = tc.alloc_tile_pool(name="tailp", bufs=1)

    def bcast_tail(name):
        row = tailp.tile([1, H], BF16, name=f"row_{name}", tag="trow", bufs=1)
        nc.gpsimd.dma_start(out=row, in_=_row(ins[name]))
        full = tailp.tile([128, H], BF16, name=f"bc_{name}", tag=name, bufs=1)
        nc.gpsimd.partition_broadcast(full, row)
        return full

    g_cn = bcast_tail("g_cn")
    b_cn = bcast_tail("b_cn")
    g_hn = bcast_tail("g_hn")
    b_hn = bcast_tail("b_hn")

    def bm_stats(zt_, bn_t):
        for s in range(SW):
            nc.vector.bn_stats(out=bn_t[:, 6 * s:6 * (s + 1)],
                               in_=zt_[:, bass.ts(s, FS)])

    for b in range(NB):
        bn_c = bnp.tile([128, 6 * SW], F32, tag=f"bn{b}", bufs=2)
        bm_stats(cp[b], bn_c)
        rstd, negm = bm_norm_cols(bn_c)
        tcell = tailp.tile([128, H], BF16, tag="tcell", bufs=2)
        for s in range(SW):
            sl = bass.ts(s, FS)
            t = tscr.tile([128, FS], BF16, tag="t", bufs=4)
            nc.scalar.activation(out=t, in_=cp[b][:, sl], func=AF.Identity,
                                 scale=rstd, bias=negm)
            nc.vector.tensor_mul(t, t, g_cn[:, sl])
            t32 = tailp.tile([128, FS], F32, tag="t32", bufs=4)
            nc.vector.tensor_add(t32, t, b_cn[:, sl])
            nc.gpsimd.dma_start(out=out_c[bass.ts(b, 128), sl], in_=t32)
            nc.scalar.activation(out=tcell[:, sl], in_=t32, func=AF.Tanh)

        # hidden: hp = o * tanh(next_cell), then LN_hn + tanh
        hp = zt["o"][b]
        nc.vector.tensor_mul(hp, hp, tcell)
        bn_h = bnp.tile([128, 6 * SW], F32, tag=f"bn{b}", bufs=2)
        bm_stats(hp, bn_h)
        rstd, negm = bm_norm_cols(bn_h)
        for s in range(SW):
            sl = bass.ts(s, FS)
            t = tscr.tile([128, FS], BF16, tag="t", bufs=4)
            nc.scalar.activation(out=t, in_=hp[:, sl], func=AF.Identity,
                                 scale=rstd, bias=negm)
            nc.vector.tensor_mul(t, t, g_hn[:, sl])
            nc.vector.tensor_add(t, t, b_hn[:, sl])
            t32 = tailp.tile([128, FS], F32, tag="t32", bufs=4)
            nc.scalar.activation(out=t32, in_=t, func=AF.Tanh)
            nc.gpsimd.dma_start(out=out_h[bass.ts(b, 128), sl], in_=t32)

    tailp.release()
    zpool.release()
    combp.release()


_NC_CACHE = {}


def _get_nc():
    if "nc" not in _NC_CACHE:
        nc = bacc.Bacc(
            "TRN2",
            target_bir_lowering=False,
            debug=False,
            enable_asserts=False,
            num_devices=NCORES,
        )
        _NC_CACHE["nc"] = build_kernel(nc)
    return _NC_CACHE["nc"]


def run(inputs, **kw):
    nc = _get_nc()
    full = {k: np.ascontiguousarray(np.asarray(v, dtype=np.float32))
            for k, v in inputs.items()}
    in_maps = []
    for i in range(NCORES):
        s = slice(i * BC, (i + 1) * BC)
        m = {k: (np.ascontiguousarray(v[s]) if k in ("x", "h", "c") else v)
             for k, v in full.items()}
        in_maps.append(m)
    res = run_bass_kernel_spmd(nc, in_maps, core_ids=list(range(NCORES)), **kw)
    nh = np.concatenate([r["out_h"] for r in res.results], axis=0)
    ncl = np.concatenate([r["out_c"] for r in res.results], axis=0)
    return np.stack([nh, ncl]).astype(np.float32), res


def kernel(**inputs) -> np.ndarray:
    out, _ = run(inputs)
    return out
